# revision 3
# baseline (speedup 1.0000x reference)
"""Trainium2 Bass kernel for nn_BinarizedLinear:
    out = sign(input_b @ sign(weight).T)
with input_b (8192, 2048) and weight (2048, 2048), entries all +/-1.0 fp32.

Since weight entries are +/-1, sign(weight) == weight; the linear output is a
sum of 2048 +/-1 terms, i.e. an even integer in [-2048, 2048], so
sign(v) == clamp(v, -1, 1) exactly, and reduced-precision matmul modes
(float32r) are bit-exact for this data.

Strategy: data-parallel across 8 NeuronCores — each core gets 1024 rows of
input_b, the full weight replicated.  Per core:
  - transpose the x-shard tile-by-tile on the TensorEngine (contraction dim
    must live on SBUF partitions for matmul), caching xT for all 8 b-tiles,
  - stream W in 512-column o-blocks: DMA + PE-transpose into SBUF, then
    matmul (float32r, full PE rate) against the cached xT, accumulating
    k=2048 in PSUM,
  - fuse sign() into the PSUM->SBUF eviction as a single tensor_scalar
    (min 1.0 then max -1.0), and DMA the fp32 result out.
"""

import numpy as np

BATCH, IN_LEN, OUT_LEN = 8192, 2048, 2048
N_CORES = 8
SHARD = BATCH // N_CORES  # 1024
P = 128

_cache = {}


def build_kernel(shard=SHARD, in_len=IN_LEN, out_len=OUT_LEN):
    import concourse.mybir as mybir
    import concourse.tile as tile
    from concourse import bacc
    from concourse.masks import make_identity

    f32 = mybir.dt.float32
    bf16 = mybir.dt.bfloat16

    KT = in_len // P          # k-tiles (contraction)
    BT = shard // P           # b-tiles per core
    OB = out_len // 512       # 512-wide output blocks
    OJ = 512 // P             # 128-row W chunks per o-block

    nc = bacc.Bacc(None, target_bir_lowering=False)
    x = nc.dram_tensor("x", [shard, in_len], f32, kind="ExternalInput")
    w = nc.dram_tensor("w", [out_len, in_len], f32, kind="ExternalInput")
    out = nc.dram_tensor("out", [shard, out_len], f32, kind="ExternalOutput")

    with tile.TileContext(nc) as tc:
        with (
            tc.tile_pool(name="const", bufs=1) as const_pool,
            tc.tile_pool(name="xt", bufs=1) as xt_pool,
            tc.tile_pool(name="stage", bufs=3) as stage_pool,
            tc.tile_pool(name="wtblk", bufs=2) as wt_pool,
            tc.tile_pool(name="outs", bufs=4) as out_pool,
            tc.tile_pool(name="tpsum", bufs=4, space="PSUM") as tpsum_pool,
            tc.tile_pool(name="mpsum", bufs=4, space="PSUM") as mpsum_pool,
        ):
            ident = const_pool.tile([P, P], f32)
            make_identity(nc, ident)

            # xT_all[p, k, b] = x[b, k*128+p] for the whole shard (bf16, exact)
            xt_all = xt_pool.tile([P, KT, shard], bf16)
            for bt in range(BT):
                xstage = stage_pool.tile([P, in_len], f32, tag="stage")
                nc.sync.dma_start(out=xstage[:], in_=x[bt * P:(bt + 1) * P, :])
                for k in range(KT):
                    tp = tpsum_pool.tile([P, P], f32)
                    nc.tensor.transpose(
                        tp[:], xstage[:, k * P:(k + 1) * P], ident[:]
                    )
                    nc.any.tensor_copy(
                        out=xt_all[:, k, bt * P:(bt + 1) * P], in_=tp[:]
                    )

            for ob in range(OB):
                # wt_blk[p, k, j*128+o] = w[ob*512 + j*128 + o, k*128+p]
                wt_blk = wt_pool.tile([P, KT, 512], bf16, tag="wtblk")
                for j in range(OJ):
                    row0 = (ob * OJ + j) * P
                    wstage = stage_pool.tile([P, in_len], f32, tag="stage")
                    nc.sync.dma_start(out=wstage[:], in_=w[row0:row0 + P, :])
                    for k in range(KT):
                        tp = tpsum_pool.tile([P, P], f32)
                        nc.tensor.transpose(
                            tp[:], wstage[:, k * P:(k + 1) * P], ident[:]
                        )
                        nc.any.tensor_copy(
                            out=wt_blk[:, k, j * P:(j + 1) * P], in_=tp[:]
                        )

                for bt in range(BT):
                    psum = mpsum_pool.tile([P, 512], f32)
                    for k in range(KT):
                        nc.tensor.matmul(
                            psum[:],
                            xt_all[:, k, bt * P:(bt + 1) * P],
                            wt_blk[:, k, :],
                            start=(k == 0),
                            stop=(k == KT - 1),
                        )
                    ot = out_pool.tile([P, 512], f32)
                    # sign(v) for integer v: clamp to [-1, 1]
                    nc.any.tensor_scalar(
                        out=ot[:], in0=psum[:], scalar1=1.0, scalar2=-1.0,
                        op0=mybir.AluOpType.min, op1=mybir.AluOpType.max,
                    )
                    nc.sync.dma_start(
                        out=out[bt * P:(bt + 1) * P, ob * 512:(ob + 1) * 512],
                        in_=ot[:],
                    )

    nc.finalize()
    return nc


def _get_nc():
    if "nc" not in _cache:
        _cache["nc"] = build_kernel()
    return _cache["nc"]


def run_sharded(input_b, weight, trace=False):
    """Run the SPMD kernel; returns (output, BassKernelResults)."""
    from concourse.bass_utils import run_bass_kernel_spmd

    nc = _get_nc()
    input_b = np.ascontiguousarray(input_b, dtype=np.float32)
    weight = np.ascontiguousarray(weight, dtype=np.float32)
    in_maps = [
        {"x": input_b[c * SHARD:(c + 1) * SHARD], "w": weight}
        for c in range(N_CORES)
    ]
    res = run_bass_kernel_spmd(nc, in_maps, list(range(N_CORES)), trace=trace)
    out = np.concatenate([res.results[c]["out"] for c in range(N_CORES)], axis=0)
    return out, res


def kernel(input_b, weight):
    out, _ = run_sharded(input_b, weight, trace=False)
    return out
